# revision 20
# baseline (speedup 1.0000x reference)
"""CFSDP (density-peaks clustering) on 8 Trainium2 NeuronCores.

Pipeline (N=8192 points, D=64, row-sharded 1024 rows/core):
  d2(i,j) = ||xi-xj||^2 via one K=68 augmented matmul per tile:
      u_i = (-2*x_i, sqh_i, sql_i, 1, 1),  v_j = (x_j, 1, 1, sqh_j, sql_j)
  All O(N^2) math runs on squared distances (sqrt is monotone, so order
  stats / argmin / percentile commute with it):
    L12 launch: ACT Tanh step-counts for 4 thresholds around the
        predicted 2%-quantile (tanh shares the exp table set -> a single
        table load; host constants are in tanh units: accum = 2c - W) ->
        on-device dc^2 interpolation -> rho via ACT Exp + accumulate.
        Inputs are per-region tiles in critical-path order (count heads
        first) so early matmuls never wait on the bulk load; per-partition
        scalars ship as [1, n] rows and are broadcast by K=1 matmuls.
    host: stable-sort rows by rho desc; per-row prefix cutoffs.
    L3 launch: delta^2 = min d2 over the sorted prefix.  The per-row
        window mask is ADDED INTO PSUM by a second accumulating matmul
        (lhsT = BIG*I, rhs = host-built 0/1 mask), so each col-group
        needs exactly one DVE min-reduce - no mask build, no extra adds.
  Host finishes: delta fallback (row max) for top-density rows, nhd argmin
  (lazy, only for non-center points), center ranks, label propagation scan.
"""

import os
import numpy as np

N = 8192
D = 64
NCORES = 8
ROWS = N // NCORES          # 1024 rows per core
P = 128                     # partitions
RB = ROWS // P              # 8 row-blocks per core
FD = 2048                   # free-dim group (4 PSUM banks)
G = N // FD                 # 4 col-groups per row
K = D + 4                   # 68 (augmented contraction dim, sq split hi+lo)
MM_N = 512                  # cols per matmul (one PSUM bank output)
MM_PER_G = FD // MM_N       # 4

NT = 4                      # percentile-count thresholds
L1_W = 256                  # cols counted per threshold
DC2_CENTER = 86.2           # chi^2_64-predicted 2%-quantile of d2 (randn data)
DC2_GRID = (DC2_CENTER * (1.0 + (np.arange(NT) - (NT - 1) / 2) * 0.023)).astype(
    np.float64
)                           # +-3.5% bracket, 2.3% spacing
PCT = 2.0
WW = 1024                   # L3 boundary mask window width
NCOL = G + 1                # L3 output cols per block (G group slots + window)
MASK_BIG = 1.0e4            # L3 mask penalty (bf16: 9984), >> max d2 ~400
EMPTY_SENTINEL = 5.0e3      # boundary min >= this => empty prefix window

# threshold b is counted on group (m, g) of every core (1/16 of the matrix
# per threshold => ~4.2M samples each; different rows+cols per threshold)
L1_GROUPS = [(b % RB, 1 + b % (G - 1)) for b in range(NT)]  # g>0: diag-free
DC2_STEP = float(DC2_CENTER * 0.023)
M_TOT = float(N) * float(N)
K_POS = PCT / 100.0 * (M_TOT - 1.0)
P_OFF = (K_POS - N) / (M_TOT - N)      # diag-free target CDF
CSTAR = float(P_OFF * P * L1_W)        # target count over the device sample
CSTAR2 = 2.0 * CSTAR                   # tanh units: accum = 2*count - L1_W
SIG_ALPHA = 2.0e4                      # sigmoid-equivalent step sharpness

_programs: dict = {}


def _build_l12():
    """Merged count + rho launch: dc^2 is computed ON DEVICE.

    Every core counts the SAME sample (rows 0..511 via the shared `uvc`
    lhsT, diag-free col groups), so each core independently derives an
    identical dc^2 - no collectives.  Counts use ACT Tanh as the step
    function (same table set as Exp -> one table load); the accum is
    2*count - L1_W, handled by tanh-unit constants (dvec, CSTAR2).
    The CDF interpolation runs as tiny
    [1,NT] vector ops; a PE ones-matmul does the partition reduction and
    a K=1 fp32 matmul broadcasts -1/dc^2 to all partitions for rho.
    `dvec` carries host-computed control-variate corrections (in counts)
    that cancel the row/col sampling bias of the fixed sample.
    """
    import concourse.mybir as mybir
    import concourse.tile as tile
    from concourse import bacc

    f32 = mybir.dt.float32
    nc = bacc.Bacc("TRN2", debug=False, enable_asserts=False)
    fp8 = mybir.dt.float8e4
    bf16 = mybir.dt.bfloat16
    UVC_W = NT * P  # only row-blocks m=0..3 are count lhsT
    h1_d = nc.dram_tensor("h1", [K, UVC_W + L1_W], fp8, kind="ExternalInput")
    h2_d = nc.dram_tensor("h2", [K, (NT - 1) * L1_W], fp8, kind="ExternalInput")
    thr_d = nc.dram_tensor("thr", [1, NT], f32, kind="ExternalInput")
    ur_d = nc.dram_tensor("ur", [K, ROWS], fp8, kind="ExternalInput")
    vt_d = [
        nc.dram_tensor(f"vt{g}", [K, FD], fp8, kind="ExternalInput")
        for g in range(G)
    ]
    tvec_d = nc.dram_tensor("tvec", [1, NT], f32, kind="ExternalInput")
    dvec_d = nc.dram_tensor("dvec", [1, NT], f32, kind="ExternalInput")
    cnt_d = nc.dram_tensor("counts", [P, NT], f32, kind="ExternalOutput")
    rho_d = nc.dram_tensor("rho", [RB, P], f32, kind="ExternalOutput")

    with tile.TileContext(nc) as tc:
        with (
            tc.tile_pool(name="inp", bufs=1) as inp,
            tc.tile_pool(name="stat", bufs=1) as stat,
            tc.tile_pool(name="trash", bufs=2) as trash_p,
            tc.tile_pool(name="psum", bufs=2, space="PSUM") as psum_p,
        ):
            # count-phase inputs on the SWDGE queue (all 16 DMA engines),
            # bulk rho inputs on the sync HWDGE queue - independent tiles
            # so each matmul waits only for its own region.
            h1_sb = inp.tile([K, UVC_W + L1_W], fp8, tag="h1")
            nc.sync.dma_start(out=h1_sb[:], in_=h1_d[:])
            h2_sb = inp.tile([K, (NT - 1) * L1_W], fp8, tag="h2")
            nc.sync.dma_start(out=h2_sb[:], in_=h2_d[:])
            uvc_sb = h1_sb[:, 0:UVC_W]
            w_ap = [h1_sb[:, UVC_W:UVC_W + L1_W]] + [
                h2_sb[:, (b - 1) * L1_W:b * L1_W] for b in range(1, NT)
            ]
            thrr_sb = inp.tile([1, NT], f32, tag="thrr")
            nc.gpsimd.dma_start(out=thrr_sb[:], in_=thr_d[:])
            tdv_sb = inp.tile([1, 2 * NT], f32, tag="tdv")
            nc.gpsimd.dma_start(out=tdv_sb[:, 0:NT], in_=tvec_d[:])
            nc.gpsimd.dma_start(out=tdv_sb[:, NT:2 * NT], in_=dvec_d[:])
            ur_sb = inp.tile([K, ROWS], fp8, tag="ur")
            nc.sync.dma_start(out=ur_sb[:], in_=ur_d[:])
            vt_sb = []
            for g in range(G):
                t = inp.tile([K, FD], fp8, tag=f"vt{g}", name=f"vt{g}_sb")
                nc.sync.dma_start(out=t[:], in_=vt_d[g][:])
                vt_sb.append(t)
            cnts = stat.tile([P, NT], f32)
            warm = stat.tile([P, 1], f32)
            nc.vector.memset(warm[:], 0.0)
            nc.scalar.activation(
                warm[:], warm[:], mybir.ActivationFunctionType.Tanh,
                bias=0.0, scale=1.0,
            )
            ones_row0 = stat.tile([1, P], f32)
            nc.vector.memset(ones_row0[:], 1.0)
            onesq = stat.tile([P, P], f32)
            nc.vector.memset(onesq[:], 1.0)
            iota_t = stat.tile([P, P], f32)
            nc.vector.tensor_tensor_scan(
                out=iota_t[:],
                data0=onesq[:],
                data1=onesq[:],
                initial=-1.0,
                op0=mybir.AluOpType.mult,
                op1=mybir.AluOpType.add,
            )
            ps_thr = psum_p.tile([P, NT + 1], f32, tag="psum")
            nc.tensor.matmul(
                ps_thr[:, 0:NT], ones_row0[:], thrr_sb[:], start=True, stop=True
            )
            one1 = stat.tile([1, 1], f32)
            nc.vector.memset(one1[:], 1.0)
            nc.tensor.matmul(
                ps_thr[:, NT:NT + 1], iota_t[0:1, 0:P], one1[:],
                start=True, stop=True,
            )
            thr_sb = stat.tile([P, NT + 1], f32)
            nc.vector.tensor_copy(thr_sb[:], ps_thr[:])
            id1_sb = stat.tile([P, P], f32, tag="id1")
            nc.vector.tensor_scalar(
                out=id1_sb[:],
                in0=iota_t[:],
                scalar1=thr_sb[:, NT:NT + 1],
                scalar2=1.0,
                op0=mybir.AluOpType.is_equal,
                op1=mybir.AluOpType.mult,
            )

            # ---- phase 1: counts over the shared sample (ACT tanh) ------
            for b, (m, g) in enumerate(L1_GROUPS):
                psum = psum_p.tile([P, FD], f32, tag="psum")
                nc.tensor.matmul(
                    psum[:, 0:L1_W],
                    uvc_sb[:, m * P:(m + 1) * P],
                    w_ap[b][:],
                    start=True,
                    stop=True,
                )
                t = trash_p.tile([P, L1_W], f32, tag="cntrash")
                nc.scalar.activation(
                    t[:],
                    psum[:, 0:L1_W],
                    mybir.ActivationFunctionType.Tanh,
                    bias=thr_sb[:, b:b + 1],
                    scale=float(-0.5 * SIG_ALPHA),
                )
                nc.vector.tensor_reduce(
                    cnts[:, b:b + 1], t[:],
                    axis=mybir.AxisListType.X, op=mybir.AluOpType.add,
                )
            nc.sync.dma_start(out=cnt_d[:], in_=cnts[:])

            # ---- phase 2: dc^2 from counts (identical on every core) ----
            ones_col = stat.tile([P, 1], f32)
            nc.vector.memset(ones_col[:], 1.0)
            ps_tot = psum_p.tile([1, NT], f32, tag="psum")
            nc.tensor.matmul(ps_tot[:], ones_col[:], cnts[:], start=True, stop=True)
            w = stat.tile([1, 8 * NT], f32)  # scratch lanes along free dim
            q = w[:, 0:NT]
            nc.vector.tensor_tensor(
                out=q, in0=ps_tot[:], in1=tdv_sb[:, NT:2 * NT],
                op=mybir.AluOpType.subtract,
            )
            NB_ = NT - 1
            a_ = w[:, NT:NT + NB_]
            nc.vector.tensor_scalar(
                out=a_, in0=q[:, 0:NB_], scalar1=CSTAR2, scalar2=None,
                op0=mybir.AluOpType.is_le,
            )
            b_ = w[:, 2 * NT:2 * NT + NB_]
            nc.vector.tensor_scalar(
                out=b_, in0=q[:, 1:NT], scalar1=CSTAR2, scalar2=None,
                op0=mybir.AluOpType.is_gt,
            )
            sel = w[:, 3 * NT:3 * NT + NB_]
            nc.vector.tensor_tensor(out=sel, in0=a_, in1=b_, op=mybir.AluOpType.mult)
            den = w[:, 4 * NT:4 * NT + NB_]
            nc.vector.tensor_tensor(
                out=den, in0=q[:, 1:NT], in1=q[:, 0:NB_],
                op=mybir.AluOpType.subtract,
            )
            rec = w[:, 5 * NT:5 * NT + NB_]
            nc.vector.reciprocal(rec, den)
            num = w[:, 6 * NT:6 * NT + NB_]
            nc.vector.tensor_scalar(
                out=num, in0=q[:, 0:NB_], scalar1=-1.0, scalar2=CSTAR2,
                op0=mybir.AluOpType.mult, op1=mybir.AluOpType.add,
            )
            fr = w[:, 7 * NT:7 * NT + NB_]
            nc.vector.tensor_tensor(out=fr, in0=num, in1=rec, op=mybir.AluOpType.mult)
            nc.vector.tensor_scalar(
                out=fr, in0=fr, scalar1=float(DC2_STEP), scalar2=None,
                op0=mybir.AluOpType.mult,
            )
            nc.vector.tensor_tensor(
                out=fr, in0=fr, in1=tdv_sb[:, 0:NB_], op=mybir.AluOpType.add
            )
            nc.vector.tensor_tensor(out=fr, in0=fr, in1=sel, op=mybir.AluOpType.mult)
            sc = stat.tile([1, 4], f32)
            nc.vector.tensor_reduce(
                sc[:, 0:1], fr[:], axis=mybir.AxisListType.X, op=mybir.AluOpType.add
            )
            nc.vector.tensor_reduce(
                sc[:, 1:2], sel[:], axis=mybir.AxisListType.X, op=mybir.AluOpType.add
            )
            # guard: if no bracket, fall back to the grid center
            nc.vector.tensor_scalar(
                out=sc[:, 2:3], in0=sc[:, 1:2], scalar1=float(-DC2_CENTER),
                scalar2=float(DC2_CENTER), op0=mybir.AluOpType.mult,
                op1=mybir.AluOpType.add,
            )
            nc.vector.tensor_tensor(
                out=sc[:, 0:1], in0=sc[:, 0:1], in1=sc[:, 2:3],
                op=mybir.AluOpType.add,
            )
            nc.vector.reciprocal(sc[:, 3:4], sc[:, 0:1])
            nc.vector.tensor_scalar(
                out=sc[:, 3:4], in0=sc[:, 3:4], scalar1=-1.0, scalar2=None,
                op0=mybir.AluOpType.mult,
            )
            ones_row = stat.tile([1, P], f32)
            nc.vector.memset(ones_row[:], 1.0)
            ps_b = psum_p.tile([P, 1], f32, tag="psum")
            nc.tensor.matmul(ps_b[:], ones_row[:], sc[:, 3:4], start=True, stop=True)
            scl_sb = stat.tile([P, 1], f32)
            nc.vector.tensor_copy(scl_sb[:], ps_b[:])

            # ---- phase 3: rho ------------------------------------------
            parts = stat.tile([P, RB * G], f32)
            rho_sb = stat.tile([P, RB], f32)
            for m in range(RB):
                for g in range(G):
                    psum = psum_p.tile([P, FD], f32, tag="psum")
                    for j in range(MM_PER_G):
                        nc.tensor.matmul(
                            psum[:, j * MM_N:(j + 1) * MM_N],
                            ur_sb[:, m * P:(m + 1) * P],
                            vt_sb[g][:, j * MM_N:(j + 1) * MM_N],
                            start=True,
                            stop=True,
                        )
                    q2 = m * G + g
                    t = trash_p.tile([P, FD], f32, tag="trash")
                    nc.scalar.activation(
                        t[:],
                        psum[:],
                        mybir.ActivationFunctionType.Exp,
                        bias=0.0,
                        scale=scl_sb[:, 0:1],
                        accum_out=parts[:, q2:q2 + 1],
                    )
                nc.vector.tensor_reduce(
                    rho_sb[:, m:m + 1],
                    parts[:, m * G:(m + 1) * G],
                    axis=mybir.AxisListType.X,
                    op=mybir.AluOpType.add,
                )
            ps_rt = psum_p.tile([RB, P], f32, tag="psum")
            nc.tensor.transpose(ps_rt[:], rho_sb[:], id1_sb[:])
            rhoT_sb = stat.tile([RB, P], f32)
            nc.vector.tensor_copy(rhoT_sb[:], ps_rt[:])
            nc.sync.dma_start(out=rho_d[:], in_=rhoT_sb[:])
    nc.compile()
    return nc


def _build_l3():
    """Delta pass on rho-sorted data (round-robin block interleaving).

    Core c holds sorted row-blocks b = 8m + c (m = 0..7).  For local block
    m: boundary col-group gb = m//2, window base w_lo = 1024*(m%2).
    Structure per block:
      groups g < gb:  plain DVE min-reduce of the [P,2048] psum.
      boundary group (cols [0, w_lo+1024)): the d2 matmuls of the window
        chunks leave the psum banks OPEN (stop=False); a second matmul
        (lhsT = MASK_BIG*I built on device, rhs = 0/1 masks built by ACT
        sigmoid from a scan-generated iota and per-row cut biases)
        accumulates the penalty, then ONE min-reduce covers prefix+window.
      columns beyond the window are never matmul'd.
    Ties that push a row's cutoff below its block's boundary window are
    patched exactly on the host (straddle_fix), as are empty prefixes
    (boundary min >= EMPTY_SENTINEL).
    """
    import concourse.mybir as mybir
    import concourse.tile as tile
    from concourse import bacc

    f32 = mybir.dt.float32
    nc = bacc.Bacc("TRN2", debug=False, enable_asserts=False)
    bf16 = mybir.dt.bfloat16
    fp8 = mybir.dt.float8e4
    ur_d = nc.dram_tensor("ur", [K, ROWS], fp8, kind="ExternalInput")
    v0a_d = nc.dram_tensor("v0a", [K, WW], fp8, kind="ExternalInput")
    v0b_d = nc.dram_tensor("v0b", [K, WW], fp8, kind="ExternalInput")
    vt_d = [
        nc.dram_tensor(f"vt{g}", [K, FD], fp8, kind="ExternalInput")
        for g in range(1, G)
    ]
    bias_d = nc.dram_tensor("biasc", [RB, P], f32, kind="ExternalInput")
    id8_d = nc.dram_tensor("id8", [RB, RB], f32, kind="ExternalInput")
    dmin_d = nc.dram_tensor("dmin", [RB * NCOL, P], f32, kind="ExternalOutput")

    with tile.TileContext(nc) as tc:
        with (
            tc.tile_pool(name="inp", bufs=1) as inp,
            tc.tile_pool(name="stat", bufs=1) as stat,
            tc.tile_pool(name="psum", bufs=2, space="PSUM") as psum_p,
        ):
            ur_sb = inp.tile([K, ROWS], fp8, tag="ur")
            nc.sync.dma_start(out=ur_sb[:], in_=ur_d[:])
            biasT_sb = inp.tile([RB, P], f32, tag="biasT")
            nc.gpsimd.dma_start(out=biasT_sb[:], in_=bias_d[:])
            id8_sb = inp.tile([RB, RB], f32, tag="id8")
            nc.gpsimd.dma_start(out=id8_sb[:], in_=id8_d[:])
            v0_sb = inp.tile([K, FD], fp8, tag="vt0")
            nc.sync.dma_start(out=v0_sb[:, 0:WW], in_=v0a_d[:])
            nc.sync.dma_start(out=v0_sb[:, WW:FD], in_=v0b_d[:])
            vt_sb = [v0_sb]
            for g in range(1, G):
                t = inp.tile([K, FD], fp8, tag=f"vt{g}", name=f"vt{g}_sb")
                nc.sync.dma_start(out=t[:], in_=vt_d[g - 1][:])
                vt_sb.append(t)
            dmin_sb = stat.tile([P, RB * NCOL], f32)
            warm = stat.tile([P, 1], f32)
            nc.vector.memset(warm[:], 0.0)
            nc.scalar.activation(
                warm[:], warm[:], mybir.ActivationFunctionType.Sigmoid,
                bias=0.0, scale=1.0,
            )
            ones_t = stat.tile([P, WW], f32)
            nc.vector.memset(ones_t[:], 1.0)
            iota_t = stat.tile([P, WW], f32)
            nc.vector.tensor_tensor_scan(
                out=iota_t[:],
                data0=ones_t[:],
                data1=ones_t[:],
                initial=-1.0,
                op0=mybir.AluOpType.mult,
                op1=mybir.AluOpType.add,
            )
            one1 = stat.tile([1, 1], f32)
            nc.vector.memset(one1[:], 1.0)
            # per-partition bias via one K=RB matmul; partition index via K=1
            ps_b = psum_p.tile([P, RB + 1], f32, tag="psum")
            nc.tensor.matmul(
                ps_b[:, 0:RB], biasT_sb[:], id8_sb[:], start=True, stop=True,
            )
            nc.tensor.matmul(
                ps_b[:, RB:RB + 1], iota_t[0:1, 0:P], one1[:],
                start=True, stop=True,
            )
            bias_sb = stat.tile([P, RB + 1], f32)
            nc.vector.tensor_copy(bias_sb[:], ps_b[:])
            # idb = MASK_BIG * I built from iota vs partition index
            id_sb = stat.tile([P, P], bf16, tag="idb")
            nc.vector.tensor_scalar(
                out=id_sb[:],
                in0=iota_t[:, 0:P],
                scalar1=bias_sb[:, RB:RB + 1],
                scalar2=MASK_BIG,
                op0=mybir.AluOpType.is_equal,
                op1=mybir.AluOpType.mult,
            )
            id1_sb = stat.tile([P, P], f32, tag="id1")
            nc.vector.tensor_scalar(
                out=id1_sb[:],
                in0=iota_t[:, 0:P],
                scalar1=bias_sb[:, RB:RB + 1],
                scalar2=1.0,
                op0=mybir.AluOpType.is_equal,
                op1=mybir.AluOpType.mult,
            )
            mask_sb = stat.tile([P, RB * WW], bf16, tag="mask")
            for m in range(RB):
                nc.scalar.activation(
                    mask_sb[:, m * WW:(m + 1) * WW],
                    iota_t[:],
                    mybir.ActivationFunctionType.Sigmoid,
                    bias=bias_sb[:, m:m + 1],
                    scale=2.0e4,
                )

            for m in range(RB):
                gb = m // 2
                w_lo = WW * (m % 2)
                bw = w_lo + WW          # matmul'd cols in the boundary group
                for g in range(gb + 1):
                    ncols = FD if g < gb else bw
                    psum = psum_p.tile([P, FD], f32, tag="psum")
                    for j in range(ncols // MM_N):
                        in_window = g == gb and j * MM_N >= w_lo
                        nc.tensor.matmul(
                            psum[:, j * MM_N:(j + 1) * MM_N],
                            ur_sb[:, m * P:(m + 1) * P],
                            vt_sb[g][:, j * MM_N:(j + 1) * MM_N],
                            start=True,
                            stop=not in_window,
                        )
                        if in_window:
                            wcol = j * MM_N - w_lo
                            nc.tensor.matmul(
                                psum[:, j * MM_N:(j + 1) * MM_N],
                                id_sb[:],
                                mask_sb[:, m * WW + wcol:m * WW + wcol + MM_N],
                                start=False,
                                stop=True,
                            )
                    q = m * NCOL + g
                    if g < gb:
                        nc.vector.tensor_reduce(
                            dmin_sb[:, q:q + 1],
                            psum[:],
                            axis=mybir.AxisListType.X,
                            op=mybir.AluOpType.min,
                        )
                    else:
                        nc.vector.tensor_reduce(
                            dmin_sb[:, m * NCOL + G:m * NCOL + G + 1],
                            psum[:, 0:bw],
                            axis=mybir.AxisListType.X,
                            op=mybir.AluOpType.min,
                        )
            ps_t = psum_p.tile([RB * NCOL, P], f32, tag="psum")
            nc.tensor.transpose(ps_t[:], dmin_sb[:], id1_sb[:])
            dmT_sb = stat.tile([RB * NCOL, P], f32)
            nc.vector.tensor_copy(dmT_sb[:], ps_t[:])
            nc.sync.dma_start(out=dmin_d[:], in_=dmT_sb[:])
    nc.compile()
    return nc


_BUILDERS = {"l12": _build_l12, "l3": _build_l3}


def _get_program(name):
    if name not in _programs:
        _programs[name] = _BUILDERS[name]()
    return _programs[name]


TIMINGS = []  # (name, exec_time_ns) per launch, appended by _run


def _run(name, in_maps, trace=None):
    from concourse.bass_utils import run_bass_kernel_spmd

    if trace is None:
        trace = bool(int(os.environ.get("KERNEL_TRACE", "0")))
    nc = _get_program(name)
    res = run_bass_kernel_spmd(
        nc, in_maps, core_ids=list(range(NCORES)), trace=trace
    )
    TIMINGS.append((name, res.exec_time_ns))
    return res


def _augmented(data):
    """U (lhs rows) and V (rhs cols) of the K=68 augmented distance GEMM.

    fp8e4 operands with sq split into an fp8 hi+lo pair: d2 error ~1.2 abs
    (~1.4% at the dc^2 scale).  Every consumer decision has >=10x margin:
    count blur ~0.01 in dc^2, rho noise ~0.2% (the pipeline is entirely
    self-consistent in its own rho), delta vs threshold ~1000x.
    """
    import ml_dtypes

    f8 = ml_dtypes.float8_e4m3fn
    sq = np.einsum("ij,ij->i", data, data, dtype=np.float32).astype(np.float32)
    sqh = sq.astype(f8)
    sql = (sq - sqh.astype(np.float32)).astype(f8)
    ones = np.ones((N, 1), f8)
    zcol = lambda a: a[:, None]
    U = np.concatenate(
        [(-2.0 * data).astype(f8), zcol(sqh), zcol(sql), ones, ones], axis=1
    )
    V = np.concatenate(
        [data.astype(f8), ones, ones, zcol(sqh), zcol(sql)], axis=1
    )
    return U, V, sq


def _erf(x):
    """Abramowitz-Stegun 7.1.26 vectorized erf (|err| < 1.5e-7)."""
    s = np.sign(x)
    x = np.abs(x)
    t = 1.0 / (1.0 + 0.3275911 * x)
    y = 1.0 - (
        ((((1.061405429 * t - 1.453152027) * t) + 1.421413741) * t - 0.284496736)
        * t
        + 0.254829592
    ) * t * np.exp(-x * x)
    return s * y


def _phi(z):
    return 0.5 * (1.0 + _erf(z / np.sqrt(2.0)))


NGRID = 256


def _cv_corrections(sq):
    """Control-variate count corrections for the fixed count sample.

    Model P(d2 < t | sq_i, sq_j) ~ Phi((t - sq_i - sq_j)/(2 sqrt(sq_i sq_j/D)))
    and subtract the predicted row/col selection bias of the sampled
    rows/cols relative to the full point set.
    """
    sq64 = sq.astype(np.float64)
    step = N // NGRID
    grid = np.sort(sq64)[step // 2::step][:NGRID]

    def h(t, svals):
        s = svals[:, None]
        sp = grid[None, :]
        z = (t - s - sp) / (2.0 * np.sqrt(np.maximum(s * sp, 1e-9) / D))
        return _phi(z).mean(axis=1)

    dvec = np.zeros(NT)
    for b, (m, g) in enumerate(L1_GROUPS):
        t = float(DC2_GRID[b])
        h_all = h(t, grid).mean()
        d_row = h(t, sq64[m * P:(m + 1) * P]).mean() - h_all
        d_col = h(t, sq64[g * FD:g * FD + L1_W]).mean() - h_all
        dvec[b] = (d_row + d_col) * (P * L1_W)
    return dvec.astype(np.float32).reshape(1, NT)


def _host_fallback(data, rho_t, delta_t):
    """Pure-numpy reference path (only used if device assumptions break)."""
    data = np.asarray(data, np.float32)
    sq = np.sum(data * data, axis=1)
    d2 = sq[:, None] + sq[None, :] - 2.0 * (data @ data.T)
    dist = np.sqrt(np.maximum(d2, 0.0), dtype=np.float32)
    dc = np.percentile(dist, PCT)
    rho = np.exp(-((dist / dc) ** 2)).sum(axis=1).astype(np.float32)
    higher = rho[None, :] > rho[:, None]
    masked = np.where(higher, dist, np.inf)
    delta_m = masked.min(axis=1)
    nhd_m = masked.argmin(axis=1)
    has = higher.any(axis=1)
    delta = np.where(has, delta_m, dist.max(axis=1))
    nhd = np.where(has, nhd_m, np.arange(N))
    return _finish_labels(rho, delta, nhd, rho_t, delta_t)


def _finish_labels(rho, delta, nhd, rho_t, delta_t):
    is_center = (rho > rho_t) & (delta > delta_t)
    center_rank = np.cumsum(is_center.astype(np.int32)) - 1
    labels = np.where(is_center, center_rank, -1).astype(np.int32)
    order = np.argsort(-rho, kind="stable")
    for i in order:
        if labels[i] < 0:
            labels[i] = labels[nhd[i]]
    return labels


def kernel(data, rho_threshold, delta_threshold):
    data = np.ascontiguousarray(np.asarray(data, dtype=np.float32))
    assert data.shape == (N, D)
    rho_t = float(np.asarray(rho_threshold))
    delta_t = float(np.asarray(delta_threshold))

    U, V, sq = _augmented(data)
    VT = np.ascontiguousarray(V.T)  # [K, N]

    # ---- L12: counts -> on-device dc^2 -> rho (single launch) ----------
    tvec = DC2_GRID.astype(np.float32).reshape(1, NT)
    dvec_sig = _cv_corrections(sq).astype(np.float64)
    dvec = (2.0 * dvec_sig - float(P * L1_W)).astype(np.float32)
    thr = (0.5 * SIG_ALPHA * DC2_GRID).astype(np.float32).reshape(1, NT)
    uvc = np.ascontiguousarray(U[0:NT * P].T)
    ws = [VT[:, g * FD:g * FD + L1_W] for b, (m, g) in enumerate(L1_GROUPS)]
    h1 = np.ascontiguousarray(np.concatenate([uvc, ws[0]], axis=1))
    h2 = np.ascontiguousarray(np.concatenate(ws[1:], axis=1))
    vt_in = {
        f"vt{g}": np.ascontiguousarray(VT[:, g * FD:(g + 1) * FD])
        for g in range(G)
    }
    in_maps = [
        {
            "h1": h1,
            "h2": h2,
            "thr": thr,
            "ur": np.ascontiguousarray(U[c * ROWS:(c + 1) * ROWS].T),
            "tvec": tvec,
            "dvec": dvec,
            **vt_in,
        }
        for c in range(NCORES)
    ]
    r12 = _run("l12", in_maps)

    # validate the on-device dc interpolation (tanh units: q = 2*(c - dvec_sig))
    q = r12.results[0]["counts"].astype(np.float64).sum(axis=0) - dvec[0].astype(
        np.float64
    )
    brackets = [b for b in range(NT - 1) if q[b] <= CSTAR2 < q[b + 1]]
    if len(brackets) != 1 or not np.all(np.diff(q) > 0):
        return _host_fallback(data, rho_t, delta_t)

    rho = np.empty(N, np.float32)
    for c in range(NCORES):
        out = r12.results[c]["rho"]  # shipped transposed: [RB, P]
        rho[c * ROWS:(c + 1) * ROWS] = out.reshape(-1)
    if not np.all(np.isfinite(rho)) or rho.min() < 0.5 or rho.max() > N + 1:
        return _host_fallback(data, rho_t, delta_t)

    # ---- host: sort by rho desc; prefix cutoffs ------------------------
    order = np.argsort(-rho, kind="stable")
    rho_sorted = rho[order]
    # c_i = #points with rho strictly greater (ties excluded)
    cuts = np.searchsorted(-rho_sorted, -rho_sorted, side="left").astype(np.int64)

    data_p = data[order]
    sq_p = sq[order]
    Up = U[order]
    Vp = V[order]
    rhs_p = np.ascontiguousarray(Vp.T)

    # round-robin block interleave: core c <- sorted blocks 8m + c
    NB = N // P  # 64 sorted row-blocks
    blk_rows = np.arange(N).reshape(NB, P)
    core_rows = [blk_rows[np.arange(RB) * NCORES + c].reshape(-1) for c in range(NCORES)]

    import ml_dtypes

    bf = ml_dtypes.bfloat16
    vt3_in = {
        f"vt{g}": np.ascontiguousarray(rhs_p[:, g * FD:(g + 1) * FD])
        for g in range(1, G)
    }
    vt3_in["v0a"] = np.ascontiguousarray(rhs_p[:, 0:WW])
    vt3_in["v0b"] = np.ascontiguousarray(rhs_p[:, WW:FD])
    in_maps = []
    for c in range(NCORES):
        rows = core_rows[c]
        biasc = np.empty((RB, P), np.float32)
        for m in range(RB):
            base = (m // 2) * FD + WW * (m % 2)
            cutrel = np.clip(cuts[rows[m * P:(m + 1) * P]] - base, 0, WW)
            biasc[m] = 2.0e4 * (0.5 - cutrel.astype(np.float64))
        in_maps.append(
            {
                "ur": np.ascontiguousarray(Up[rows].T),
                "biasc": biasc,
                "id8": np.eye(RB, dtype=np.float32),
                **vt3_in,
            }
        )
    r3 = _run("l3", in_maps)
    # dmin[i] holds per-source minima; slot k < gb = full group k,
    # slot G = boundary group (prefix + masked window in one reduce)
    dmin = np.full((N, NCOL), np.inf, np.float32)
    for c in range(NCORES):
        out = r3.results[c]["dmin"].T  # shipped transposed: [RB*NCOL, P]
        rows = core_rows[c]
        for m in range(RB):
            gb = m // 2
            blk = rows[m * P:(m + 1) * P]
            for g in range(gb):
                dmin[blk, g] = out[:, m * NCOL + g]
            dmin[blk, G] = out[:, m * NCOL + G]

    # ---- host: delta, fallback rows, centers, nhd (lazy), labels -------
    delta2_sorted = dmin.min(axis=1)

    # rho-tie rows whose cutoff dips below their block's boundary window:
    # the device's unmasked prefix included a few extra columns; fix exactly.
    win_base = ((np.arange(N) // P) // NCORES) * WW  # 1024*m per sorted row
    straddle_fix = {}
    for i in np.nonzero(cuts < win_base)[0]:
        cut = int(cuts[i])
        if cut == 0:
            delta2_sorted[i] = np.inf
            continue
        d2row = sq_p[i] + sq_p[:cut] - 2.0 * (data_p[:cut] @ data_p[i])
        j = int(np.argmin(d2row))
        delta2_sorted[i] = d2row[j]
        straddle_fix[i] = j

    empty = delta2_sorted >= EMPTY_SENTINEL  # no higher-density point
    delta_sorted = np.sqrt(np.maximum(delta2_sorted, 0.0), dtype=np.float32)
    for i in np.nonzero(empty)[0]:
        d2row = sq_p[i] + sq_p - 2.0 * (data_p @ data_p[i])
        delta_sorted[i] = np.sqrt(max(float(np.max(np.maximum(d2row, 0.0))), 0.0))

    delta = np.empty(N, np.float32)
    delta[order] = delta_sorted

    is_center = (rho > rho_t) & (delta > delta_t)
    center_rank = np.cumsum(is_center.astype(np.int32)) - 1
    labels = np.where(is_center, center_rank, -1).astype(np.int32)

    need_nhd = ~is_center[order]  # sorted positions whose label must propagate
    nhd = np.arange(N, dtype=np.int64)  # default: self (matches reference)
    for i in np.nonzero(need_nhd)[0]:
        if empty[i]:
            continue  # nhd stays self, as in reference
        if i in straddle_fix:
            nhd[order[i]] = order[straddle_fix[i]]
            continue
        k = int(np.argmin(dmin[i]))
        m = (i // P) // NCORES
        gb = m // 2
        w_lo = WW * (m % 2)
        if k == G:
            c0, clen = gb * FD, w_lo + WW
        else:
            c0, clen = k * FD, FD
        end_local = int(np.clip(cuts[i] - c0, 0, clen))
        cols = slice(c0, c0 + end_local)
        d2part = sq_p[i] + sq_p[cols] - 2.0 * (data_p[cols] @ data_p[i])
        j_local = int(np.argmin(d2part))
        nhd[order[i]] = order[c0 + j_local]

    for i in order:
        if labels[i] < 0:
            labels[i] = labels[nhd[i]]
    return labels.astype(np.int32)
